# revision 13
# baseline (speedup 1.0000x reference)
"""Segment mean-pool (BERT lattice embedding) Trainium2 Bass kernel.

Full-input contract: kernel(hidden[64,512,768] f32, word_ids[64,512] i32,
num_tokens=400) -> [64,400,768] f32.

Strategy: data-parallel over batch across 8 NeuronCores (8 samples each).
Per sample b the ragged segment mean  out[t] = mean_{s: wid[s]==t} hidden[s]
is computed as a matmul on the PE array with the MEAN WEIGHTS folded into the
one-hot matrix:

    A[s, t]   = (word_ids[b, s] == t) / count[b, word_ids[b, s]]
    psum[h,t] = sum_j hid[b, j-chunk, h].T @ A[j-chunk, t]
    out[h, t] = psum[h, t]                      (plain PSUM->SBUF copy)

Layout choices vs the previous version:
  - [h, t] output orientation: stationary operand = hid chunk [128s x 128h],
    moving operand = A chunk [128s x 400t].  Every matmul uses the full 128
    partitions and full 128 stationary columns; the ragged T=400 lands in the
    free dim.  PE streaming cycles drop from J*ceil(T/128)*H = 12288 to
    J*(H/128)*T = 9600 per sample, and the mw=16 tail chunk is gone.
  - fp16 end-to-end on the heavy tensors (hidden in, pooled out).  Host casts
    (dtype/layout transforms only - no arithmetic on activations).  Halves
    HBM traffic: 22.4 MB -> 11.2 MB per core.  Values are O(1) means of
    N(0,1), so fp16 keeps ~5e-4 relative error (gate is 2e-2).
  - per-piece weight w[s] = 1/count[wid[s]] is a per-partition scalar, so the
    one-hot build is ONE DVE tensor_scalar (is_equal then mult) per (b, j),
    and the PSUM eviction needs no scaling at all.

The output leaves the device as out[b, g, p, t] = pooled[b, t, 128g+p]
(h-major); the host transposes back to [B, T, H] (index-side work only).

DMA ring assignment: inputs prefetch on the sync HWDGE ring (entire shard up
front - fits SBUF), outputs stream on the scalar HWDGE ring.
"""

import numpy as np

B, S, H, T = 64, 512, 768, 400
N_CORES = 8
B_LOC = B // N_CORES  # samples per core
P = 128
J = S // P  # contraction chunks per sample
G = H // P  # output h-groups per sample

_CACHED = {}


def _windows(wid):
    """Static per-chunk word windows [L_j, R_j) from the sorted word_ids.

    Chunk j holds pieces s in [128j, 128j+128); their words span a narrow
    band.  Union over ALL samples in the batch -> one program serves all
    cores.  Any word outside every window has zero pieces batch-wide, so its
    output rows are zero (host fills them).
    """
    wid = np.asarray(wid, np.int64).reshape(B, S)
    win = [(0, T)]  # chunk 0 always full-width: its start=True matmul must
    # initialize the whole PSUM bank (writes exact zeros where it has no
    # pieces), so chunks 1+ are pure accumulates into written bytes.
    for j in range(1, J):
        lo = int(wid[:, j * P].min())
        hi = int(wid[:, j * P + P - 1].max()) + 1
        win.append((lo, hi))
    return tuple(win)


def build_program(windows=((0, T),) * J):
    """Build + compile the single-core Bass program (same NEFF on all cores)."""
    import concourse.bass as bass  # noqa: F401
    import concourse.mybir as mybir
    import concourse.tile as tile
    from concourse import bacc

    nc = bacc.Bacc(
        "TRN2",
        target_bir_lowering=False,
        debug=False,
        enable_asserts=False,
        num_devices=N_CORES,
    )
    f32 = mybir.dt.float32
    f16 = mybir.dt.float16

    # hidden host-prearranged as [B_LOC, P, J, H] fp16:
    # hid_pjh[b, p, j, h] = hidden[b, 128j + p, h] -> the per-sample DMA is one
    # fully linear 786 KB transfer with 6 KB/partition contiguous runs.
    hidden_t = nc.dram_tensor(
        "hidden_pjh", [B_LOC, P, J, H], f16, kind="ExternalInput"
    ).ap()
    # wid_pbj[p, b, j] = word_ids[b, 128j+p] as fp32 (the tensor_scalar
    # per-partition scalar operands must be fp32).
    wid_t = nc.dram_tensor("wid_pbj", [P, B_LOC, J], f32, kind="ExternalInput").ap()
    # w_pbj[p, b, j] = 1/count[b, word_ids[b, 128j+p]] - the per-piece mean
    # weight (host-computed from the 128 KB index tensor).
    w_t = nc.dram_tensor("w_pbj", [P, B_LOC, J], f32, kind="ExternalInput").ap()
    # out[b, p, g, t] = pooled[b, t, 128g+p] fp16; host transposes back.
    out_t = nc.dram_tensor("out", [B_LOC, P, G, T], f16, kind="ExternalOutput").ap()

    GB = G // 2  # h-groups per output DMA batch

    with tile.TileContext(nc) as tc:
        with tc.tile_pool(name="const", bufs=1) as const_pool, \
             tc.tile_pool(name="hidp", bufs=B_LOC) as hid_pool, \
             tc.tile_pool(name="aTp", bufs=3) as aT_pool, \
             tc.tile_pool(name="outp", bufs=4) as out_pool, \
             tc.tile_pool(name="psum", bufs=8, space="PSUM") as psum_pool:

            # All one-hot-build operands fp16 (16-bit DVE fast path); values
            # are small integers / reciprocals, exactly representable.
            iota_t = const_pool.tile([P, T], f16, name="iota_t")
            nc.gpsimd.iota(
                iota_t,
                pattern=[[1, T]],
                base=0,
                channel_multiplier=0,
                allow_small_or_imprecise_dtypes=True,
            )

            # Tiny index tensors first: they gate the aT builds.
            wid_sb = const_pool.tile([P, B_LOC, J], f32, name="wid_sb")
            nc.sync.dma_start(out=wid_sb, in_=wid_t)
            w_sb = const_pool.tile([P, B_LOC, J], f32, name="w_sb")
            nc.sync.dma_start(out=w_sb, in_=w_t)

            # Prefetch the whole input shard up front (fits in SBUF): 8 x
            # 786 KB back-to-back on the input ring.  Sample 0 split per
            # j-chunk so its first matmuls can start earlier.
            hids = []
            for b in range(B_LOC):
                hid = hid_pool.tile([P, J, H], f16, name=f"hid{b}", tag="hid")
                if b == 0:
                    for j in range(J):
                        nc.sync.dma_start(out=hid[:, j, :], in_=hidden_t[b][:, j, :])
                else:
                    nc.sync.dma_start(out=hid, in_=hidden_t[b])
                hids.append(hid)

            for b in range(B_LOC):
                hid = hids[b]
                aT = aT_pool.tile([P, J, T], f16, name="aT", tag="aT")
                for j in range(J):
                    lo, hi = windows[j]
                    # aT[p, j, t] = (iota[t] == wid[b, 128j+p]) * w[b, 128j+p]
                    # Built (and later streamed) only over the chunk's word
                    # window - everything outside is structurally zero.
                    nc.vector.tensor_scalar(
                        aT[:, j, lo:hi],
                        iota_t[:, lo:hi],
                        wid_sb[:, b, j : j + 1],
                        w_sb[:, b, j : j + 1],
                        op0=mybir.AluOpType.is_equal,
                        op1=mybir.AluOpType.mult,
                    )
                for half in range(G // GB):
                    om = out_pool.tile([P, GB, T], f16, name="om", tag="om")
                    for k in range(GB):
                        g = half * GB + k
                        ps = psum_pool.tile([P, T], f32, name="ps", tag="ps")
                        for j in range(J):
                            lo, hi = windows[j]
                            # j=0 start=True clears has_written for the whole
                            # bank; later chunks overwrite where the bit is
                            # clear and accumulate where set, so the narrow
                            # windows compose correctly.  Columns outside all
                            # windows are zero-filled by the host.
                            nc.tensor.matmul(
                                ps[:, lo:hi],
                                hid[:, j, g * P : (g + 1) * P],
                                aT[:, j, lo:hi],
                                start=(j == 0),
                                stop=(j == J - 1),
                            )
                        # Plain PSUM->SBUF eviction (mean already applied via
                        # w).  Alternate ACT/DVE so neither engine bottlenecks.
                        if g % 2 == 0:
                            nc.vector.tensor_copy(om[:, k, :], ps)
                        else:
                            nc.scalar.copy(om[:, k, :], ps)
                    # Batched output DMA (3 h-groups = 300 KB, contiguous on
                    # both sides) on the scalar HWDGE ring: 16 issues total
                    # instead of 48 - HWDGE descriptor-gen runs on the issuing
                    # engine, so fewer/bigger DMAs keep ACT free for copies.
                    nc.scalar.dma_start(
                        out=out_t[b, :, half * GB : (half + 1) * GB, :], in_=om
                    )

    nc.compile()
    return nc


def _prep_in_maps(hidden, word_ids):
    hidden = np.ascontiguousarray(np.asarray(hidden), dtype=np.float32).reshape(B, S, H)
    wid = np.ascontiguousarray(np.asarray(word_ids), dtype=np.int32).reshape(B, S)

    # Per-word piece counts -> per-piece mean weight 1/count[wid[s]].
    counts = np.zeros((B, T), np.int64)
    rows = np.repeat(np.arange(B), S)
    np.add.at(counts, (rows, wid.reshape(-1)), 1)
    recip = (1.0 / np.maximum(counts, 1)).astype(np.float32)  # [B, T]
    wpiece = np.take_along_axis(recip, wid, axis=1)  # [B, S]

    in_maps = []
    for i in range(N_CORES):
        sl = slice(i * B_LOC, (i + 1) * B_LOC)
        # [B_LOC, S, H] -> [B_LOC, P, J, H] with s = 128j + p, cast fp16.
        hs = hidden[sl].reshape(B_LOC, J, P, H).transpose(0, 2, 1, 3)
        hs = np.ascontiguousarray(hs, dtype=np.float16)
        # [B_LOC, S] -> [P, B_LOC, J]
        wj = np.ascontiguousarray(
            wid[sl].reshape(B_LOC, J, P).transpose(2, 0, 1).astype(np.float32)
        )
        wp = np.ascontiguousarray(
            wpiece[sl].reshape(B_LOC, J, P).transpose(2, 0, 1).astype(np.float32)
        )
        in_maps.append({"hidden_pjh": hs, "wid_pbj": wj, "w_pbj": wp})
    return in_maps


def _zero_uncovered(out, windows):
    """Zero word rows outside every chunk window (zero pieces batch-wide)."""
    covered = np.zeros(T, bool)
    for lo, hi in windows:
        covered[lo:hi] = True
    out[:, ~covered, :] = 0.0
    return out


def run(hidden, word_ids, trace=False, **trace_kwargs):
    from concourse import bass_utils

    windows = _windows(word_ids)
    if _CACHED.get("windows") != windows:
        _CACHED["nc"] = build_program(windows)
        _CACHED["windows"] = windows
    nc = _CACHED["nc"]
    in_maps = _prep_in_maps(hidden, word_ids)
    res = bass_utils.run_bass_kernel_spmd(
        nc, in_maps, core_ids=list(range(N_CORES)), trace=trace, **trace_kwargs
    )
    # [N_CORES x [B_LOC, P, G, T]] -> [B, T, H] fp32 with h = 128g + p.
    out = np.concatenate([np.asarray(res.results[i]["out"]) for i in range(N_CORES)])
    out = out.transpose(0, 3, 2, 1).reshape(B, T, H).astype(np.float32)
    return _zero_uncovered(np.ascontiguousarray(out), windows), res


def kernel(hidden, word_ids, num_tokens=None, **_unused):
    out, _ = run(hidden, word_ids, trace=False)
    return out


# revision 17
# speedup vs baseline: 1.0190x; 1.0190x over previous
"""Segment mean-pool (BERT lattice embedding) Trainium2 Bass kernel.

Full-input contract: kernel(hidden[64,512,768] f32, word_ids[64,512] i32,
num_tokens=400) -> [64,400,768] f32.

Strategy: data-parallel over batch across 8 NeuronCores (8 samples each).
Per sample b the ragged segment mean  out[t] = mean_{s: wid[s]==t} hidden[s]
is computed as a matmul on the PE array with the MEAN WEIGHTS folded into the
one-hot matrix:

    A[s, t]   = (word_ids[b, s] == t) / count[b, word_ids[b, s]]
    psum[h,t] = sum_j hid[b, j-chunk, h].T @ A[j-chunk, t]
    out[h, t] = psum[h, t]                      (plain PSUM->SBUF copy)

Layout choices vs the previous version:
  - [h, t] output orientation: stationary operand = hid chunk [128s x 128h],
    moving operand = A chunk [128s x 400t].  Every matmul uses the full 128
    partitions and full 128 stationary columns; the ragged T=400 lands in the
    free dim.  PE streaming cycles drop from J*ceil(T/128)*H = 12288 to
    J*(H/128)*T = 9600 per sample, and the mw=16 tail chunk is gone.
  - fp16 end-to-end on the heavy tensors (hidden in, pooled out).  Host casts
    (dtype/layout transforms only - no arithmetic on activations).  Halves
    HBM traffic: 22.4 MB -> 11.2 MB per core.  Values are O(1) means of
    N(0,1), so fp16 keeps ~5e-4 relative error (gate is 2e-2).
  - per-piece weight w[s] = 1/count[wid[s]] is a per-partition scalar, so the
    one-hot build is ONE DVE tensor_scalar (is_equal then mult) per (b, j),
    and the PSUM eviction needs no scaling at all.

The output leaves the device as out[b, g, p, t] = pooled[b, t, 128g+p]
(h-major); the host transposes back to [B, T, H] (index-side work only).

DMA ring assignment: inputs prefetch on the sync HWDGE ring (entire shard up
front - fits SBUF), outputs stream on the scalar HWDGE ring.
"""

import numpy as np

B, S, H, T = 64, 512, 768, 400
N_CORES = 8
B_LOC = B // N_CORES  # samples per core
P = 128
J = S // P  # contraction chunks per sample
G = H // P  # output h-groups per sample

_CACHED = {}


def _windows(wid):
    """Static per-chunk word windows [L_j, R_j) from the sorted word_ids.

    Chunk j holds pieces s in [128j, 128j+128); their words span a narrow
    band.  Union over ALL samples in the batch -> one program serves all
    cores.  Any word outside every window has zero pieces batch-wide, so its
    output rows are zero (host fills them).
    """
    wid = np.asarray(wid, np.int64).reshape(B, S)
    win = [(0, T)]  # chunk 0 always full-width: its start=True matmul must
    # initialize the whole PSUM bank (writes exact zeros where it has no
    # pieces), so chunks 1+ are pure accumulates into written bytes.
    for j in range(1, J):
        lo = int(wid[:, j * P].min())
        hi = int(wid[:, j * P + P - 1].max()) + 1
        win.append((lo, hi))
    return tuple(win)


def build_program(windows=((0, T),) * J):
    """Build + compile the single-core Bass program (same NEFF on all cores)."""
    import concourse.bass as bass  # noqa: F401
    import concourse.mybir as mybir
    import concourse.tile as tile
    from concourse import bacc

    nc = bacc.Bacc(
        "TRN2",
        target_bir_lowering=False,
        debug=False,
        enable_asserts=False,
        num_devices=N_CORES,
    )
    f32 = mybir.dt.float32
    f16 = mybir.dt.float16

    # hidden host-prearranged as [B_LOC, P, J, H] fp16:
    # hid_pjh[b, p, j, h] = hidden[b, 128j + p, h] -> the per-sample DMA is one
    # fully linear 786 KB transfer with 6 KB/partition contiguous runs.
    hidden_t = nc.dram_tensor(
        "hidden_pjh", [B_LOC, P, J, H], f16, kind="ExternalInput"
    ).ap()
    # wid_pbj[p, b, j] = word_ids[b, 128j+p] as fp32 (the tensor_scalar
    # per-partition scalar operands must be fp32).
    wid_t = nc.dram_tensor("wid_pbj", [P, B_LOC, J], f32, kind="ExternalInput").ap()
    # w_pbj[p, b, j] = 1/count[b, word_ids[b, 128j+p]] - the per-piece mean
    # weight (host-computed from the 128 KB index tensor).
    w_t = nc.dram_tensor("w_pbj", [P, B_LOC, J], f32, kind="ExternalInput").ap()
    # out[b, p, g, t] = pooled[b, t, 128g+p] fp16; host transposes back.
    out_t = nc.dram_tensor("out", [B_LOC, P, G, T], f16, kind="ExternalOutput").ap()
    # Scratch sink for the HAM warm-up matmuls (keeps them from being DCE'd).
    warm_t = nc.dram_tensor("warm_out", [P, T], f16, kind="ExternalOutput").ap()

    GB = G // 2  # h-groups per output DMA batch

    with tile.TileContext(nc) as tc:
        with tc.tile_pool(name="const", bufs=1) as const_pool, \
             tc.tile_pool(name="hidp", bufs=B_LOC) as hid_pool, \
             tc.tile_pool(name="aTp", bufs=4) as aT_pool, \
             tc.tile_pool(name="outp", bufs=8) as out_pool, \
             tc.tile_pool(name="psum", bufs=8, space="PSUM") as psum_pool:

            # All one-hot-build operands fp16 (16-bit DVE fast path); values
            # are small integers / reciprocals, exactly representable.
            iota_t = const_pool.tile([P, T], f16, name="iota_t")
            nc.gpsimd.iota(
                iota_t,
                pattern=[[1, T]],
                base=0,
                channel_multiplier=0,
                allow_small_or_imprecise_dtypes=True,
            )

            # Tiny index tensors first: they gate the aT builds.
            wid_sb = const_pool.tile([P, B_LOC, J], f32, name="wid_sb")
            nc.sync.dma_start(out=wid_sb, in_=wid_t)
            w_sb = const_pool.tile([P, B_LOC, J], f32, name="w_sb")
            nc.sync.dma_start(out=w_sb, in_=w_t)

            # Prefetch the whole input shard up front (fits in SBUF): 8 x
            # 786 KB back-to-back on the input ring (the g-outer loop needs a
            # full sample before its first matmul group completes, so one DMA
            # per sample beats j-chunking: fewer ~650 ns HWDGE issues).
            hids = []
            for b in range(B_LOC):
                hid = hid_pool.tile([P, J, H], f16, name=f"hid{b}", tag="hid")
                nc.sync.dma_start(out=hid, in_=hidden_t[b])
                hids.append(hid)

            # HAM warm-up: the PE clock-gate defaults to 4/8 (1.2 GHz) and
            # only reaches 8/8 after ~3.4 us of sustained matmul activity.
            # Burn dummy matmuls on the iota tile while the first sample's
            # input is still in flight, so the real matmuls start warm.
            wps = psum_pool.tile([P, T], f32, name="wps", tag="ps")
            for _ in range(8):
                nc.tensor.matmul(
                    wps, iota_t[:, :P], iota_t, start=True, stop=True
                )
            wsb = const_pool.tile([P, T], f16, name="wsb")
            # x0 scale: iota.T @ iota values overflow fp16; the sink only
            # exists to keep the warm-up matmuls live.
            nc.vector.tensor_scalar_mul(wsb, wps, 0.0)
            nc.scalar.dma_start(out=warm_t, in_=wsb)

            for b in range(B_LOC):
                hid = hids[b]
                aT = aT_pool.tile([P, J, T], f16, name="aT", tag="aT")
                for j in range(J):
                    lo, hi = windows[j]
                    # aT[p, j, t] = (iota[t] == wid[b, 128j+p]) * w[b, 128j+p]
                    # Built (and later streamed) only over the chunk's word
                    # window - everything outside is structurally zero.
                    nc.vector.tensor_scalar(
                        aT[:, j, lo:hi],
                        iota_t[:, lo:hi],
                        wid_sb[:, b, j : j + 1],
                        w_sb[:, b, j : j + 1],
                        op0=mybir.AluOpType.is_equal,
                        op1=mybir.AluOpType.mult,
                    )
                for half in range(G // GB):
                    om = out_pool.tile([P, GB, T], f16, name="om", tag="om")
                    for k in range(GB):
                        g = half * GB + k
                        ps = psum_pool.tile([P, T], f32, name="ps", tag="ps")
                        for j in range(J):
                            lo, hi = windows[j]
                            # j=0 start=True clears has_written for the whole
                            # bank; later chunks overwrite where the bit is
                            # clear and accumulate where set, so the narrow
                            # windows compose correctly.  Columns outside all
                            # windows are zero-filled by the host.
                            nc.tensor.matmul(
                                ps[:, lo:hi],
                                hid[:, j, g * P : (g + 1) * P],
                                aT[:, j, lo:hi],
                                start=(j == 0),
                                stop=(j == J - 1),
                            )
                        # Plain PSUM->SBUF eviction (mean already applied via
                        # w).  Alternate ACT/DVE so neither engine bottlenecks.
                        if g % 2 == 0:
                            nc.vector.tensor_copy(om[:, k, :], ps)
                        else:
                            nc.scalar.copy(om[:, k, :], ps)
                    # Batched output DMA (3 h-groups = 300 KB, contiguous on
                    # both sides) on the scalar HWDGE ring: 16 issues total
                    # instead of 48 - HWDGE descriptor-gen runs on the issuing
                    # engine, so fewer/bigger DMAs keep ACT free for copies.
                    nc.scalar.dma_start(
                        out=out_t[b, :, half * GB : (half + 1) * GB, :], in_=om
                    )

    nc.compile()
    return nc


def _prep_in_maps(hidden, word_ids):
    hidden = np.ascontiguousarray(np.asarray(hidden), dtype=np.float32).reshape(B, S, H)
    wid = np.ascontiguousarray(np.asarray(word_ids), dtype=np.int32).reshape(B, S)

    # Per-word piece counts -> per-piece mean weight 1/count[wid[s]].
    counts = np.zeros((B, T), np.int64)
    rows = np.repeat(np.arange(B), S)
    np.add.at(counts, (rows, wid.reshape(-1)), 1)
    recip = (1.0 / np.maximum(counts, 1)).astype(np.float32)  # [B, T]
    wpiece = np.take_along_axis(recip, wid, axis=1)  # [B, S]

    in_maps = []
    for i in range(N_CORES):
        sl = slice(i * B_LOC, (i + 1) * B_LOC)
        # [B_LOC, S, H] -> [B_LOC, P, J, H] with s = 128j + p, cast fp16.
        hs = hidden[sl].reshape(B_LOC, J, P, H).transpose(0, 2, 1, 3)
        hs = np.ascontiguousarray(hs, dtype=np.float16)
        # [B_LOC, S] -> [P, B_LOC, J]
        wj = np.ascontiguousarray(
            wid[sl].reshape(B_LOC, J, P).transpose(2, 0, 1).astype(np.float32)
        )
        wp = np.ascontiguousarray(
            wpiece[sl].reshape(B_LOC, J, P).transpose(2, 0, 1).astype(np.float32)
        )
        in_maps.append({"hidden_pjh": hs, "wid_pbj": wj, "w_pbj": wp})
    return in_maps


def _zero_uncovered(out, windows):
    """Zero word rows outside every chunk window (zero pieces batch-wide)."""
    covered = np.zeros(T, bool)
    for lo, hi in windows:
        covered[lo:hi] = True
    out[:, ~covered, :] = 0.0
    return out


def run(hidden, word_ids, trace=False, **trace_kwargs):
    from concourse import bass_utils

    windows = _windows(word_ids)
    if _CACHED.get("windows") != windows:
        _CACHED["nc"] = build_program(windows)
        _CACHED["windows"] = windows
    nc = _CACHED["nc"]
    in_maps = _prep_in_maps(hidden, word_ids)
    res = bass_utils.run_bass_kernel_spmd(
        nc, in_maps, core_ids=list(range(N_CORES)), trace=trace, **trace_kwargs
    )
    # [N_CORES x [B_LOC, P, G, T]] -> [B, T, H] fp32 with h = 128g + p.
    out = np.concatenate([np.asarray(res.results[i]["out"]) for i in range(N_CORES)])
    out = out.transpose(0, 3, 2, 1).reshape(B, T, H).astype(np.float32)
    return _zero_uncovered(np.ascontiguousarray(out), windows), res


def kernel(hidden, word_ids, num_tokens=None, **_unused):
    out, _ = run(hidden, word_ids, trace=False)
    return out


# revision 22
# speedup vs baseline: 1.1709x; 1.1491x over previous
"""Segment mean-pool (BERT lattice embedding) Trainium2 Bass kernel.

Full-input contract: kernel(hidden[64,512,768] f32, word_ids[64,512] i32,
num_tokens=400) -> [64,400,768] f32.

Strategy: data-parallel over batch across 8 NeuronCores (8 samples each).
Per sample b the ragged segment mean  out[t] = mean_{s: wid[s]==t} hidden[s]
is computed as a matmul on the PE array with the MEAN WEIGHTS folded into the
one-hot matrix:

    A[s, t]   = (word_ids[b, s] == t) / count[b, word_ids[b, s]]
    psum[h,t] = sum_j hid[b, j-chunk, h].T @ A[j-chunk, t]
    out[h, t] = psum[h, t]                      (plain PSUM->SBUF copy)

Layout choices vs the previous version:
  - [h, t] output orientation: stationary operand = hid chunk [128s x 128h],
    moving operand = A chunk [128s x 400t].  Every matmul uses the full 128
    partitions and full 128 stationary columns; the ragged T=400 lands in the
    free dim.  PE streaming cycles drop from J*ceil(T/128)*H = 12288 to
    J*(H/128)*T = 9600 per sample, and the mw=16 tail chunk is gone.
  - fp16 end-to-end on the heavy tensors (hidden in, pooled out).  Host casts
    (dtype/layout transforms only - no arithmetic on activations).  Halves
    HBM traffic: 22.4 MB -> 11.2 MB per core.  Values are O(1) means of
    N(0,1), so fp16 keeps ~5e-4 relative error (gate is 2e-2).
  - per-piece weight w[s] = 1/count[wid[s]] is a per-partition scalar, so the
    one-hot build is ONE DVE tensor_scalar (is_equal then mult) per (b, j),
    and the PSUM eviction needs no scaling at all.

The output leaves the device as out[b, g, p, t] = pooled[b, t, 128g+p]
(h-major); the host transposes back to [B, T, H] (index-side work only).

DMA ring assignment: inputs prefetch on the sync HWDGE ring (entire shard up
front - fits SBUF), outputs stream on the scalar HWDGE ring.
"""

import numpy as np

B, S, H, T = 64, 512, 768, 400
N_CORES = 8
B_LOC = B // N_CORES  # samples per core
P = 128
J = S // P  # contraction chunks per sample
G = H // P  # output h-groups per sample

_CACHED = {}


def _windows(wid):
    """Static per-chunk word windows [L_j, R_j) from the sorted word_ids.

    Chunk j holds pieces s in [128j, 128j+128); their words span a narrow
    band.  Union over ALL samples in the batch -> one program serves all
    cores.  Any word outside every window has zero pieces batch-wide, so its
    output rows are zero (host fills them).
    """
    wid = np.asarray(wid, np.int64).reshape(B, S)
    win = [(0, T)]  # chunk 0 always full-width: its start=True matmul must
    # initialize the whole PSUM bank (writes exact zeros where it has no
    # pieces), so chunks 1+ are pure accumulates into written bytes.
    for j in range(1, J):
        lo = int(wid[:, j * P].min())
        hi = int(wid[:, j * P + P - 1].max()) + 1
        win.append((lo, hi))
    return tuple(win)


def build_program(windows=((0, T),) * J):
    """Build + compile the single-core Bass program (same NEFF on all cores)."""
    import concourse.bass as bass  # noqa: F401
    import concourse.mybir as mybir
    import concourse.tile as tile
    from concourse import bacc

    nc = bacc.Bacc(
        "TRN2",
        target_bir_lowering=False,
        debug=False,
        enable_asserts=False,
        num_devices=N_CORES,
    )
    f32 = mybir.dt.float32
    f16 = mybir.dt.float16

    # hidden host-prearranged as [B_LOC, P, J, H] fp16:
    # hid_pjh[b, p, j, h] = hidden[b, 128j + p, h] -> the per-sample DMA is one
    # fully linear 786 KB transfer with 6 KB/partition contiguous runs.
    hidden_t = nc.dram_tensor(
        "hidden_pjh", [B_LOC, P, J, H], f16, kind="ExternalInput"
    ).ap()
    # wid_pbj[p, b, j] = word_ids[b, 128j+p] as fp32 (the tensor_scalar
    # per-partition scalar operands must be fp32).
    wid_t = nc.dram_tensor("wid_pbj", [P, B_LOC, J], f32, kind="ExternalInput").ap()
    # w_pbj[p, b, j] = 1/count[b, word_ids[b, 128j+p]] - the per-piece mean
    # weight (host-computed from the 128 KB index tensor).
    w_t = nc.dram_tensor("w_pbj", [P, B_LOC, J], f32, kind="ExternalInput").ap()
    # out[b, p, g, t] = pooled[b, t, 128g+p] fp16; host transposes back.
    out_t = nc.dram_tensor("out", [B_LOC, P, G, T], f16, kind="ExternalOutput").ap()
    # Scratch sink for the HAM warm-up matmuls (keeps them from being DCE'd).
    warm_t = nc.dram_tensor("warm_out", [P, 16], f16, kind="ExternalOutput").ap()

    GB = G // 2  # h-groups per output DMA batch

    with tile.TileContext(nc) as tc:
        with tc.tile_pool(name="const", bufs=1) as const_pool, \
             tc.tile_pool(name="hidp", bufs=B_LOC) as hid_pool, \
             tc.tile_pool(name="aTp", bufs=4) as aT_pool, \
             tc.tile_pool(name="outp", bufs=2 * G) as out_pool, \
             tc.tile_pool(name="psum", bufs=8, space="PSUM") as psum_pool:

            # All one-hot-build operands fp16 (16-bit DVE fast path); values
            # are small integers / reciprocals, exactly representable.
            iota_t = const_pool.tile([P, T], f16, name="iota_t")
            nc.gpsimd.iota(
                iota_t,
                pattern=[[1, T]],
                base=0,
                channel_multiplier=0,
                allow_small_or_imprecise_dtypes=True,
            )

            # Prefetch the whole input shard up front (fits in SBUF): 8 x
            # 786 KB back-to-back on the sync ring.  ALL DMAs - inputs first,
            # then outputs as they are produced - go on this ONE ring: the
            # ring drains transfers in issue order, so the input prefetch gets
            # the full HBM bandwidth (output transfers would otherwise
            # round-robin at packet granularity and halve the input rate,
            # starving the matmul pipeline; total bytes are HBM-bound either
            # way, so input-then-output ordering is optimal).
            hids = []
            for b in range(B_LOC):
                hid = hid_pool.tile([P, J, H], f16, name=f"hid{b}", tag="hid")
                nc.sync.dma_start(out=hid, in_=hidden_t[b])
                hids.append(hid)
                if b == 0:
                    # Tiny index tensors slot in right behind sample 0.
                    wid_sb = const_pool.tile([P, B_LOC, J], f32, name="wid_sb")
                    nc.sync.dma_start(out=wid_sb, in_=wid_t)
                    w_sb = const_pool.tile([P, B_LOC, J], f32, name="w_sb")
                    nc.sync.dma_start(out=w_sb, in_=w_t)

            # HAM warm-up: the PE clock-gate defaults to 4/8 (1.2 GHz) and
            # only reaches 8/8 after ~3.4 us of sustained matmul activity.
            # Burn dummy matmuls on the iota tile while the first sample's
            # input is still in flight, so the real matmuls start warm.
            wps = psum_pool.tile([P, T], f32, name="wps", tag="ps")
            for _ in range(8):
                nc.tensor.matmul(
                    wps, iota_t[:, :P], iota_t, start=True, stop=True
                )
            wsb = const_pool.tile([P, 16], f16, name="wsb")
            # x0 scale: iota.T @ iota values overflow fp16; the sink only
            # exists to keep the warm-up matmuls live.
            nc.vector.tensor_scalar_mul(wsb, wps[:, :16], 0.0)
            nc.scalar.dma_start(out=warm_t, in_=wsb)

            for b in range(B_LOC):
                hid = hids[b]
                aT = aT_pool.tile([P, J, T], f16, name="aT", tag="aT")
                for j in range(J):
                    lo, hi = windows[j]
                    # aT[p, j, t] = (iota[t] == wid[b, 128j+p]) * w[b, 128j+p]
                    # Built (and later streamed) only over the chunk's word
                    # window - everything outside is structurally zero.
                    nc.vector.tensor_scalar(
                        aT[:, j, lo:hi],
                        iota_t[:, lo:hi],
                        wid_sb[:, b, j : j + 1],
                        w_sb[:, b, j : j + 1],
                        op0=mybir.AluOpType.is_equal,
                        op1=mybir.AluOpType.mult,
                    )
                for half in range(G // GB):
                    om = out_pool.tile([P, GB, T], f16, name="om", tag="om")
                    for k in range(GB):
                        g = half * GB + k
                        ps = psum_pool.tile([P, T], f32, name="ps", tag="ps")
                        for j in range(J):
                            lo, hi = windows[j]
                            # j=0 start=True clears has_written for the whole
                            # bank; later chunks overwrite where the bit is
                            # clear and accumulate where set, so the narrow
                            # windows compose correctly.  Columns outside all
                            # windows are zero-filled by the host.
                            nc.tensor.matmul(
                                ps[:, lo:hi],
                                hid[:, j, g * P : (g + 1) * P],
                                aT[:, j, lo:hi],
                                start=(j == 0),
                                stop=(j == J - 1),
                            )
                        # Plain PSUM->SBUF eviction (mean already applied via
                        # w).  DVE also builds aT, so give ACT 2 of every 3.
                        if g % 3 == 0:
                            nc.vector.tensor_copy(om[:, k, :], ps)
                        else:
                            nc.scalar.copy(om[:, k, :], ps)
                    # Batched output DMA (3 h-groups = 300 KB, contiguous on
                    # both sides) on the SAME sync ring as the input prefetch:
                    # queued behind it in FIFO order (see prefetch comment).
                    nc.sync.dma_start(
                        out=out_t[b, :, half * GB : (half + 1) * GB, :], in_=om
                    )

    nc.compile()
    return nc


def _prep_in_maps(hidden, word_ids):
    hidden = np.ascontiguousarray(np.asarray(hidden), dtype=np.float32).reshape(B, S, H)
    wid = np.ascontiguousarray(np.asarray(word_ids), dtype=np.int32).reshape(B, S)

    # Per-word piece counts -> per-piece mean weight 1/count[wid[s]].
    counts = np.zeros((B, T), np.int64)
    rows = np.repeat(np.arange(B), S)
    np.add.at(counts, (rows, wid.reshape(-1)), 1)
    recip = (1.0 / np.maximum(counts, 1)).astype(np.float32)  # [B, T]
    wpiece = np.take_along_axis(recip, wid, axis=1)  # [B, S]

    in_maps = []
    for i in range(N_CORES):
        sl = slice(i * B_LOC, (i + 1) * B_LOC)
        # [B_LOC, S, H] -> [B_LOC, P, J, H] with s = 128j + p, cast fp16.
        hs = hidden[sl].reshape(B_LOC, J, P, H).transpose(0, 2, 1, 3)
        hs = np.ascontiguousarray(hs, dtype=np.float16)
        # [B_LOC, S] -> [P, B_LOC, J]
        wj = np.ascontiguousarray(
            wid[sl].reshape(B_LOC, J, P).transpose(2, 0, 1).astype(np.float32)
        )
        wp = np.ascontiguousarray(
            wpiece[sl].reshape(B_LOC, J, P).transpose(2, 0, 1).astype(np.float32)
        )
        in_maps.append({"hidden_pjh": hs, "wid_pbj": wj, "w_pbj": wp})
    return in_maps


def _zero_uncovered(out, windows):
    """Zero word rows outside every chunk window (zero pieces batch-wide)."""
    covered = np.zeros(T, bool)
    for lo, hi in windows:
        covered[lo:hi] = True
    out[:, ~covered, :] = 0.0
    return out


def run(hidden, word_ids, trace=False, **trace_kwargs):
    from concourse import bass_utils

    windows = _windows(word_ids)
    if _CACHED.get("windows") != windows:
        _CACHED["nc"] = build_program(windows)
        _CACHED["windows"] = windows
    nc = _CACHED["nc"]
    in_maps = _prep_in_maps(hidden, word_ids)
    res = bass_utils.run_bass_kernel_spmd(
        nc, in_maps, core_ids=list(range(N_CORES)), trace=trace, **trace_kwargs
    )
    # [N_CORES x [B_LOC, P, G, T]] -> [B, T, H] fp32 with h = 128g + p.
    out = np.concatenate([np.asarray(res.results[i]["out"]) for i in range(N_CORES)])
    out = out.transpose(0, 3, 2, 1).reshape(B, T, H).astype(np.float32)
    return _zero_uncovered(np.ascontiguousarray(out), windows), res


def kernel(hidden, word_ids, num_tokens=None, **_unused):
    out, _ = run(hidden, word_ids, trace=False)
    return out


# revision 23
# speedup vs baseline: 1.2903x; 1.1020x over previous
"""Segment mean-pool (BERT lattice embedding) Trainium2 Bass kernel.

Full-input contract: kernel(hidden[64,512,768] f32, word_ids[64,512] i32,
num_tokens=400) -> [64,400,768] f32.

Strategy: data-parallel over batch across 8 NeuronCores (8 samples each).
Per sample b the ragged segment mean  out[t] = mean_{s: wid[s]==t} hidden[s]
is computed as a matmul on the PE array with the MEAN WEIGHTS folded into the
one-hot matrix:

    A[s, k]   = (wid'[s] == k) / count[wid[s]]      (k = compact word rank)
    psum[h,k] = sum_j hid[j-chunk, h].T @ A[j-chunk, k]
    out[h, k] = psum[h, k]                          (plain PSUM->SBUF copy)

Key layout/precision choices (each measured on HW):
  - fp16 end-to-end on the heavy tensors: halves HBM traffic; values are O(1)
    means of N(0,1) so rel-err stays ~4e-4 (gate 2e-2).
  - [h, k] output orientation: stationary operand = hid chunk [128s x 128h],
    moving = one-hot [128s x W].  Full partition utilization, no ragged tail.
  - COMPACT word axis: word_ids are sorted; only ~290 of 400 words per sample
    have pieces.  Host remaps each sample's words to ranks [0, n_b) and the
    device works at static width W = max_b n_b (rounded up).  Host scatters
    rows back (index-side work).  ~24% fewer output bytes + narrower psum
    evictions and one-hot builds.
  - WINDOWED matmuls: piece-chunk j only touches a narrow compact-rank band.
    Chunk 0 runs full-width (its start=True must initialize the whole PSUM
    bank - writes exact zeros where it has no pieces); chunks 1-3 stream only
    their windows (pure accumulates into written bytes).
  - ONE DMA ring (sync) for input prefetch AND outputs, in program order:
    the ring drains FIFO, so the input prefetch gets full HBM bandwidth and
    outputs stream right behind - total bytes are HBM-bound either way, so
    input-first is the optimal schedule.  om pool is deep enough (2G) that
    evictions never block on output drains.
  - HAM warm-up matmuls at t~8.5us (PE clock-gate sits at 1.2 GHz until
    ~3.4us of sustained activity); their sink eviction is emitted AFTER the
    main loop on ACT so it cannot delay the first one-hot builds.
"""

import numpy as np

B, S, H, T = 64, 512, 768, 400
N_CORES = 8
B_LOC = B // N_CORES  # samples per core
P = 128
J = S // P  # contraction chunks per sample
G = H // P  # output h-groups per sample
GB = G // 2  # h-groups per output DMA batch

_CACHED = {}


def _prep_meta(word_ids):
    """Host index-side preprocessing of the 128 KB word_ids tensor.

    Returns (W, windows, uniq, wid_compact):
      - uniq[s]: sorted present words of sample s (np.unique)
      - wid_compact[s, i]: rank of word_ids[s, i] within uniq[s]
      - W: static compact width = max_s len(uniq[s]) rounded up to 8
      - windows[j]: compact-rank window [lo, hi) of piece-chunk j, union over
        ALL samples (one program serves all cores); windows[0] = (0, W).
    """
    wid = np.asarray(word_ids, np.int64).reshape(B, S)
    uniq = []
    wid_c = np.empty_like(wid)
    for s in range(B):
        u, inv = np.unique(wid[s], return_inverse=True)
        uniq.append(u)
        wid_c[s] = inv
    W = int(np.ceil(max(len(u) for u in uniq) / 8) * 8)
    windows = [(0, W)]
    for j in range(1, J):
        lo = int(wid_c[:, j * P].min())
        hi = int(wid_c[:, j * P + P - 1].max()) + 1
        windows.append((lo, hi))
    return W, tuple(windows), uniq, wid_c


def build_program(W, windows):
    """Build + compile the single-core Bass program (same NEFF on all cores)."""
    import concourse.bass as bass  # noqa: F401
    import concourse.mybir as mybir
    import concourse.tile as tile
    from concourse import bacc

    nc = bacc.Bacc(
        "TRN2",
        target_bir_lowering=False,
        debug=False,
        enable_asserts=False,
        num_devices=N_CORES,
    )
    f32 = mybir.dt.float32
    f16 = mybir.dt.float16

    # hidden host-prearranged as [B_LOC, P, J, H] fp16:
    # hid_pjh[b, p, j, h] = hidden[b, 128j + p, h] -> the per-sample DMA is one
    # fully linear 786 KB transfer with 6 KB/partition contiguous runs.
    hidden_t = nc.dram_tensor(
        "hidden_pjh", [B_LOC, P, J, H], f16, kind="ExternalInput"
    ).ap()
    # wid_pbj[p, b, j] = compact rank of piece 128j+p, fp32 (tensor_scalar
    # per-partition scalar operands must be fp32).
    wid_t = nc.dram_tensor("wid_pbj", [P, B_LOC, J], f32, kind="ExternalInput").ap()
    # w_pbj[p, b, j] = 1/count[word of piece 128j+p] - per-piece mean weight.
    w_t = nc.dram_tensor("w_pbj", [P, B_LOC, J], f32, kind="ExternalInput").ap()
    # out[b, p, g, k] = pooled'[b, k, 128g+p] fp16; host scatters back.
    out_t = nc.dram_tensor("out", [B_LOC, P, G, W], f16, kind="ExternalOutput").ap()
    # Scratch sink for the HAM warm-up matmuls (keeps them from being DCE'd).
    warm_t = nc.dram_tensor("warm_out", [P, 16], f16, kind="ExternalOutput").ap()

    with tile.TileContext(nc) as tc:
        with tc.tile_pool(name="const", bufs=1) as const_pool, \
             tc.tile_pool(name="hidp", bufs=B_LOC) as hid_pool, \
             tc.tile_pool(name="aTp", bufs=4) as aT_pool, \
             tc.tile_pool(name="outp", bufs=2 * G) as out_pool, \
             tc.tile_pool(name="psum", bufs=8, space="PSUM") as psum_pool:

            # One-hot-build operands: iota fp16 (16-bit DVE path; compact
            # ranks < 2048 are exact in fp16).
            iota_t = const_pool.tile([P, W], f16, name="iota_t")
            nc.gpsimd.iota(
                iota_t,
                pattern=[[1, W]],
                base=0,
                channel_multiplier=0,
                allow_small_or_imprecise_dtypes=True,
            )

            # Prefetch the whole input shard up front (fits in SBUF): 8 x
            # 786 KB back-to-back.  ALL DMAs - inputs first, then outputs as
            # they are produced - go on the ONE sync ring: it drains in issue
            # order, so the prefetch gets full HBM bandwidth (outputs would
            # otherwise round-robin at packet granularity and halve the input
            # rate, starving the matmul pipeline).
            hids = []
            for b in range(B_LOC):
                hid = hid_pool.tile([P, J, H], f16, name=f"hid{b}", tag="hid")
                nc.sync.dma_start(out=hid, in_=hidden_t[b])
                hids.append(hid)
                if b == 0:
                    # Tiny index tensors slot in right behind sample 0.
                    wid_sb = const_pool.tile([P, B_LOC, J], f32, name="wid_sb")
                    nc.sync.dma_start(out=wid_sb, in_=wid_t)
                    w_sb = const_pool.tile([P, B_LOC, J], f32, name="w_sb")
                    nc.sync.dma_start(out=w_sb, in_=w_t)

            # HAM warm-up: the PE clock-gate defaults to 4/8 (1.2 GHz) and
            # only reaches 8/8 after ~3.4 us of sustained matmul activity.
            # Burn a few dummy matmuls while sample 0 is still in flight.
            wps = psum_pool.tile([P, W], f32, name="wps", tag="ps")
            for _ in range(5):
                nc.tensor.matmul(wps, iota_t[:, :P], iota_t, start=True, stop=True)

            for b in range(B_LOC):
                hid = hids[b]
                aT = aT_pool.tile([P, J, W], f16, name="aT", tag="aT")
                for j in range(J):
                    lo, hi = windows[j]
                    # aT[p, j, k] = (iota[k] == wid'[b, 128j+p]) * w[b, 128j+p]
                    # built only over the chunk's compact-rank window.
                    nc.vector.tensor_scalar(
                        aT[:, j, lo:hi],
                        iota_t[:, lo:hi],
                        wid_sb[:, b, j : j + 1],
                        w_sb[:, b, j : j + 1],
                        op0=mybir.AluOpType.is_equal,
                        op1=mybir.AluOpType.mult,
                    )
                for half in range(G // GB):
                    om = out_pool.tile([P, GB, W], f16, name="om", tag="om")
                    for k in range(GB):
                        g = half * GB + k
                        ps = psum_pool.tile([P, W], f32, name="ps", tag="ps")
                        for j in range(J):
                            lo, hi = windows[j]
                            # j=0 (full width) start=True clears has_written
                            # for the whole bank and writes zeros where it has
                            # no pieces; j>=1 accumulate inside their windows.
                            nc.tensor.matmul(
                                ps[:, lo:hi],
                                hid[:, j, g * P : (g + 1) * P],
                                aT[:, j, lo:hi],
                                start=(j == 0),
                                stop=(j == J - 1),
                            )
                        # Plain PSUM->SBUF eviction (mean already applied via
                        # w).  DVE also builds aT, so give ACT 2 of every 3.
                        if g % 3 == 0:
                            nc.vector.tensor_copy(om[:, k, :], ps)
                        else:
                            nc.scalar.copy(om[:, k, :], ps)
                    # Batched output DMA (3 h-groups, contiguous both sides)
                    # on the SAME sync ring, FIFO behind the input prefetch.
                    nc.sync.dma_start(
                        out=out_t[b, :, half * GB : (half + 1) * GB, :], in_=om
                    )

            # Warm-up sink, emitted LAST so it cannot delay the aT builds
            # (ACT + scalar ring are otherwise idle at the end).  x0 scale:
            # iota.T @ iota values overflow fp16.
            wsb = const_pool.tile([P, 16], f16, name="wsb")
            nc.scalar.mul(wsb, wps[:, :16], 0.0)
            nc.scalar.dma_start(out=warm_t, in_=wsb)

    nc.compile()
    return nc


def _prep_in_maps(hidden, wid_c, wpiece):
    hidden = np.ascontiguousarray(np.asarray(hidden), dtype=np.float32).reshape(B, S, H)
    in_maps = []
    for i in range(N_CORES):
        sl = slice(i * B_LOC, (i + 1) * B_LOC)
        # [B_LOC, S, H] -> [B_LOC, P, J, H] with s = 128j + p, cast fp16.
        hs = hidden[sl].reshape(B_LOC, J, P, H).transpose(0, 2, 1, 3)
        hs = np.ascontiguousarray(hs, dtype=np.float16)
        # [B_LOC, S] -> [P, B_LOC, J]
        wj = np.ascontiguousarray(
            wid_c[sl].reshape(B_LOC, J, P).transpose(2, 0, 1).astype(np.float32)
        )
        wp = np.ascontiguousarray(
            wpiece[sl].reshape(B_LOC, J, P).transpose(2, 0, 1).astype(np.float32)
        )
        in_maps.append({"hidden_pjh": hs, "wid_pbj": wj, "w_pbj": wp})
    return in_maps


def _piece_weights(word_ids):
    """1/count[wid[s]] per piece, from the index tensor only."""
    wid = np.ascontiguousarray(np.asarray(word_ids), dtype=np.int64).reshape(B, S)
    counts = np.zeros((B, T), np.int64)
    np.add.at(counts, (np.repeat(np.arange(B), S), wid.reshape(-1)), 1)
    recip = (1.0 / np.maximum(counts, 1)).astype(np.float32)
    return np.take_along_axis(recip, wid, axis=1)  # [B, S]


def run(hidden, word_ids, trace=False, **trace_kwargs):
    from concourse import bass_utils

    W, windows, uniq, wid_c = _prep_meta(word_ids)
    key = (W, windows)
    if _CACHED.get("key") != key:
        _CACHED["nc"] = build_program(W, windows)
        _CACHED["key"] = key
    nc = _CACHED["nc"]
    in_maps = _prep_in_maps(hidden, wid_c, _piece_weights(word_ids))
    res = bass_utils.run_bass_kernel_spmd(
        nc, in_maps, core_ids=list(range(N_CORES)), trace=trace, **trace_kwargs
    )
    # [N_CORES x [B_LOC, P, G, W]] -> scatter compact ranks back to [B, T, H].
    dev = np.concatenate([np.asarray(res.results[i]["out"]) for i in range(N_CORES)])
    dev = dev.transpose(0, 3, 2, 1).reshape(B, W, H).astype(np.float32)  # [B, W, H]
    out = np.zeros((B, T, H), np.float32)
    for s in range(B):
        u = uniq[s]
        out[s, u, :] = dev[s, : len(u), :]
    return out, res


def kernel(hidden, word_ids, num_tokens=None, **_unused):
    out, _ = run(hidden, word_ids, trace=False)
    return out
